# revision 20
# baseline (speedup 1.0000x reference)
"""CrossAttention (cosine-sim, learnable temperature) Trainium2 kernel, v3.

Math (per batch element b, reference in fp32):
    qh  = (q @ Wq.T)   -> [Lq, C] -> heads [H, Lq, D]
    k,v = (kv @ Wkv.T) -> k,v [H, Lkv, D]
    qn = qh / ||qh||_d; kn = k / ||k||_d
    attn = softmax(qn @ kn.T / tau); out = attn @ v
    y = out @ Wproj.T + bproj         (bproj added on host)

Distribution: pure data-parallel over B=8 across the 8 NeuronCores (one
batch element per core, weights replicated, no collectives).

v3 changes (vs v2), driven by the NTFF profile (phase 2 ACT-bound at
21.4us/pair of Exp; 135us LDWEIGHTS; 28us DMA lead-in; Exp table load
at the phase boundary triggering a 37us HAM re-throttle):
  * Both heads of a pair share ONE 4-bank PSUM scores tile and ONE
    bf16 pt tile, so the softmax Exp runs at N=2048 instead of 2x
    N=1024 - 64 ACTIVATEs instead of 128, amortizing the ~480ns
    per-instruction overhead (~20% of phase-2 ACT time).  The scores
    tile is single-buffered; the interleaved PV/filler matmuls cover
    the Exp latency between kt steps.
  * Norm chain: fast-reciprocal FIRST on the tiny [2,CH] sum-of-squares
    row (DVE time is free-dim bound, partition-count independent), ACT
    Sqrt(x * 1/tau^2) gives the reciprocal norm directly, and an idle
    DMA engine broadcasts it across partitions (source AP with a
    0-stride middle free dim - partition dim stays size 1).  Kills the
    32 PE broadcast matmuls + psB bank; DVE count per job unchanged.
  * pv_tail: same trick - reciprocal of the softmax sums on the [1,CH]
    PSUM row, DMA-broadcast to [64,CH], one tensor_mul.  Kills the 32
    bf16 CAST copies (~26us DVE), 32 broadcast matmuls, and the psBc
    bank; 3-stage software pipeline (mms / recip+dma / mul) hides the
    DMA latency under the next unit's matmuls.
  * A zero-scale dummy Exp right after phase 1 preloads the ACT
    exp table set during the phase boundary, so the first real Exp
    doesn't stall scores (the v2 trace shows that stall tipping the
    PE into a 37us HAM re-throttle covering pairs 0-1).
  * Score matmuls emitted stationary-major (LDWEIGHTS dedupe probe).
  * kv-ch0 input DMAs issued before the wk columns (first K job needs
    all kv chunks but only one wk column) to cut the 28us DMA lead-in.
  * O-proj filler spread over 3 partial stages (ct 0-2 at pairs 2-4,
    2-4 at 5-6, 4-6 during pair 7); the ct 6-8 finals interleave into
    pair 7's PV steps, shrinking the ACT-idle tail.
"""

import sys

sys.path.insert(0, "/opt/trn_rl_repo")

import numpy as np
import ml_dtypes

import concourse.bass as bass
import concourse.bacc as bacc
import concourse.mybir as mybir
from concourse.tile import TileContext
from concourse.bass_utils import run_bass_kernel_spmd

AF = mybir.ActivationFunctionType
F32 = mybir.dt.float32
F32R = mybir.dt.float32r
F16 = mybir.dt.float16
BF16 = mybir.dt.bfloat16

NCORES = 8


def r32(ap):
    """fp32 AP -> float32r view (full-rate PE matmul on fp32 data)."""
    return ap.bitcast(F32R)


DEFAULT_KNOBS = dict(
    psA_bufs=5, psS_bufs=3,
    sq_bufs=3, smalls_bufs=4, rb_bufs=3,
    psSc_bufs=1, psPV_bufs=3,
    pt_bufs=2, rsum_bufs=2, sbb_bufs=3, tmp_bufs=2, y_bufs=2,
)


def build_nc(C=1024, H=16, LQ=1024, LKV=1024, knobs=None):
    kb = dict(DEFAULT_KNOBS)
    if knobs:
        kb.update(knobs)
    P = 128
    D = C // H            # head dim (64)
    OT = C // P           # feature tiles (8)
    CT = C // P           # contraction tiles (8)
    KT = LKV // P         # lkv partition tiles (8)
    HPT = P // D          # heads per 128-tile (2)
    CH = min(512, LQ)     # free-dim chunk per psum bank (fp32)
    NCH = LQ // CH        # chunks of Lq (2)
    VCH = min(512, C)     # chunk of output features for V projection
    NVCH = C // VCH
    HPC = VCH // D        # heads per v-projection chunk (8)

    nc = bacc.Bacc("TRN2", target_bir_lowering=False)

    qT = nc.dram_tensor("qT", [C, LQ], F16, kind="ExternalInput")
    kvT = nc.dram_tensor("kvT", [C, LKV], F16, kind="ExternalInput")
    wqT = nc.dram_tensor("wqT", [C, C], F16, kind="ExternalInput")
    wkT = nc.dram_tensor("wkT", [C, C], F16, kind="ExternalInput")
    wvT = nc.dram_tensor("wvT", [C, C], F16, kind="ExternalInput")
    wpT = nc.dram_tensor("wpT", [C, C], BF16, kind="ExternalInput")
    tau2 = nc.dram_tensor("tau2", [HPT, 1], F32, kind="ExternalInput")
    ones_blk = nc.dram_tensor("ones_blk", [P, HPT], F16, kind="ExternalInput")
    blk2 = nc.dram_tensor("blk2", [HPT, P], F16, kind="ExternalInput")
    y = nc.dram_tensor("y", [LQ, C], F32, kind="ExternalOutput")

    qT_r = qT.rearrange("(ct p) l -> p ct l", p=P)
    kvT_r = kvT.rearrange("(ct p) l -> p ct l", p=P)
    wqT_r = wqT.rearrange("(ct p) o -> p ct o", p=P)
    wkT_r = wkT.rearrange("(ct p) o -> p ct o", p=P)
    wvT_r = wvT.rearrange("(ct p) o -> p ct o", p=P)
    wpT_r = wpT.rearrange("(ct p) o -> p ct o", p=P)
    y_r = y.rearrange("(yt p) o -> p yt o", p=P)

    with TileContext(nc) as tc:
        from contextlib import ExitStack

        with ExitStack() as stk:
            # ---------- persistent pools --------------------------------
            persist = stk.enter_context(tc.tile_pool(name="persist", bufs=1))
            qnT = persist.tile([P, OT, LQ], F16)            # qh * rq
            knT = persist.tile([P, OT, LKV], F16)           # kh * rk / tau
            v_aug = persist.tile([P, KT, H, D + 1], BF16)   # [v | ones]
            oT = persist.tile([P, CT, LQ], BF16)            # (attn@v)/sum
            wp_sb = persist.tile([P, CT, C], BF16)
            consts = stk.enter_context(tc.tile_pool(name="consts", bufs=1))
            ones_blk_sb = consts.tile([P, HPT], F16)
            tau2_sb = consts.tile([HPT, 1], F32)
            scr = consts.tile([HPT, 1], F32)

            nc.sync.dma_start(out=ones_blk_sb, in_=ones_blk[:, :])
            nc.sync.dma_start(out=tau2_sb, in_=tau2[:, :])
            nc.vector.memset(v_aug[:, :, :, D : D + 1], 1.0)

            # ---------- phase 1 (scoped so pools free before phase 2) ----
            p1 = ExitStack()
            # kv ch0 chunks first (the first K job needs all of them but
            # only one wk column), then wk/wv columns, kv ch1, then q + q
            # weights (phase 1b), O-proj weights last.
            p1w = p1.enter_context(tc.tile_pool(name="p1w", bufs=1))
            kvT_sb = p1w.tile([P, CT, LKV], F16)
            wk_sb = p1w.tile([P, CT, C], F16)
            wv_sb = p1w.tile([P, CT, VCH], F16)   # first half only (vch 0)
            qT_sb = p1w.tile([P, CT, LQ], F16)
            wq_sb = p1w.tile([P, CT, C], F16)
            for ct in range(CT):
                nc.sync.dma_start(out=kvT_sb[:, ct, 0:CH], in_=kvT_r[:, ct, 0:CH])
            for ct in range(CT):
                sl = slice(ct * P, (ct + 1) * P)
                nc.sync.dma_start(out=wk_sb[:, :, sl], in_=wkT_r[:, :, sl])
            for ct in range(VCH // P):
                sl = slice(ct * P, (ct + 1) * P)
                nc.sync.dma_start(out=wv_sb[:, :, sl], in_=wvT_r[:, :, sl])
            for ct in range(CT):
                nc.sync.dma_start(
                    out=kvT_sb[:, ct, CH:LKV], in_=kvT_r[:, ct, CH:LKV]
                )
            for ct in range(CT):
                sl = slice(ct * P, (ct + 1) * P)
                nc.sync.dma_start(out=qT_sb[:, ct, :], in_=qT_r[:, ct, :])
                nc.sync.dma_start(out=wq_sb[:, :, sl], in_=wqT_r[:, :, sl])
            for ct in range(CT):
                nc.sync.dma_start(out=wp_sb[:, ct, :], in_=wpT_r[:, ct, :])

            # ============ PHASE 1a: K norm-proj + V proj ================
            class Job:
                def A(self):
                    pass

                def B(self):
                    pass

                def Cs(self):
                    pass

            def run_pipeline(jobs):
                n = len(jobs)
                for i in range(n + 2):
                    if i < n:
                        jobs[i].A()
                    if 0 <= i - 1 < n:
                        jobs[i - 1].B()
                    if 0 <= i - 2 < n:
                        jobs[i - 2].Cs()

            with ExitStack() as p1c:
                sqp = p1c.enter_context(tc.tile_pool(name="sqp", bufs=kb["sq_bufs"]))
                smalls = p1c.enter_context(
                    tc.tile_pool(name="smalls", bufs=kb["smalls_bufs"])
                )
                rbp = p1c.enter_context(tc.tile_pool(name="rbp", bufs=kb["rb_bufs"]))
                psA = p1c.enter_context(
                    tc.tile_pool(name="psA", bufs=kb["psA_bufs"], space="PSUM")
                )
                psS = p1c.enter_context(
                    tc.tile_pool(name="psS", bufs=kb["psS_bufs"], space="PSUM")
                )

                class NormJob(Job):
                    """Shared K/Q norm-projection job body."""

                    def __init__(self, ot, ch):
                        self.ot, self.ch = ot, ch
                        self.sl = slice(ch * CH, (ch + 1) * CH)

                    def A(self):
                        self.ph = psA.tile([P, CH], F32, tag="ph", name="ph")
                        w_sb, x_sb = self.srcs()
                        wcol = w_sb[:, :, self.ot * P : (self.ot + 1) * P]
                        for ct in range(CT):
                            nc.tensor.matmul(
                                self.ph,
                                wcol[:, ct, :],
                                x_sb[:, ct, self.sl],
                                start=(ct == 0),
                                stop=(ct == CT - 1),
                            )
                        self.sq = sqp.tile([P, CH], F16, tag="sq", name="sq")
                        nc.scalar.activation(self.sq, self.ph, AF.Square)

                    def B(self):
                        ssq = psS.tile([HPT, CH], F32, tag="ssq", name="ssq")
                        nc.tensor.matmul(ssq, ones_blk_sb, self.sq, start=True, stop=True)
                        # rr = sqrt((1/ssq) * scale) = reciprocal norm, f16.
                        # Fast-recip on the [2,CH] row costs the same as on
                        # [128,CH] (free-dim bound), so take it before the
                        # broadcast; the broadcast is then a DMA, not a PE
                        # ones-matmul.
                        rssq = smalls.tile([HPT, CH], F32, tag="rssq", name="rssq")
                        nc.vector.reciprocal_approx_fast(rssq, ssq)
                        self.rr = smalls.tile([HPT, CH], F16, tag="rr", name="rr")
                        nc.scalar.activation(
                            self.rr, rssq, AF.Sqrt, scale=self.rsqrt_scale()
                        )
                        # Broadcast on the Act HWDGE queue: the trigger sits
                        # right after the Sqrt on the same engine (no
                        # cross-engine wait) and the transfer rides a
                        # different hardware queue than the bulk input loads
                        # on qSP (which would otherwise serialize the jobs).
                        self.rb = rbp.tile([P, CH], F16, tag="rb", name="rb")
                        nc.scalar.dma_start(
                            out=self.rb[0:D, :],
                            in_=self.rr[0:1, None, :].to_broadcast((1, D, CH)),
                        )
                        nc.scalar.dma_start(
                            out=self.rb[D:P, :],
                            in_=self.rr[1:2, None, :].to_broadcast((1, D, CH)),
                        )

                    def Cs(self):
                        nc.vector.tensor_mul(
                            self.dst()[:, self.ot, self.sl], self.ph, self.rb
                        )

                class KJob(NormJob):
                    def srcs(self):
                        return wk_sb, kvT_sb

                    def rsqrt_scale(self):
                        return tau2_sb

                    def dst(self):
                        return knT

                class QJob(NormJob):
                    def srcs(self):
                        return wq_sb, qT_sb

                    def rsqrt_scale(self):
                        return 1.0

                    def dst(self):
                        return qnT

                class VJob(Job):
                    def __init__(self, vch, vt):
                        self.vch, self.vt = vch, vt

                    def A(self):
                        self.pv = psA.tile([P, VCH], F32, tag="ph", name="pv")
                        wcol = wv_sb[:, :, self.vch * VCH : (self.vch + 1) * VCH]
                        for ct in range(CT):
                            nc.tensor.matmul(
                                self.pv,
                                kvT_sb[:, ct, self.vt * P : (self.vt + 1) * P],
                                wcol[:, ct, :],
                                start=(ct == 0),
                                stop=(ct == CT - 1),
                            )

                    def Cs(self):
                        nc.vector.tensor_copy(
                            v_aug[
                                :, self.vt, self.vch * HPC : (self.vch + 1) * HPC, 0:D
                            ],
                            self.pv.rearrange("p (h d) -> p h d", d=D),
                        )

                # One merged pipeline: ch0 K jobs first (they only need the
                # ch0 kv halves), V jobs slotted in as their inputs land,
                # then ch1 K jobs, then all Q jobs.  A single pool scope
                # means no pipeline drain at the K/Q boundary.
                jobs = [KJob(ot, 0) for ot in range(4)]
                for i in range(4):
                    jobs += [KJob(4 + i, 0), VJob(0, i)]
                for i in range(4):
                    jobs += [KJob(i, 1), VJob(0, 4 + i)]
                jobs += [KJob(4 + i, 1) for i in range(4)]
                jobs += [QJob(i // 2, i % 2) for i in range(2 * OT)]
                run_pipeline(jobs)

            # Preload the ACT exp table set during the phase boundary so the
            # first real Exp doesn't stall the (single-buffered) scores tile.
            nc.scalar.activation(scr, tau2_sb, AF.Exp, scale=0.0)

            # free phase-1 inputs/weights before the big pt pool allocates
            p1.close()

            # ============ PHASE 2: attention (head pairs) ===============
            with ExitStack() as p2:
                ymp = p2.enter_context(tc.tile_pool(name="ymp", bufs=1))
                y_mid = ymp.tile([P, LQ // P, C], BF16)
                wv1p = p2.enter_context(tc.tile_pool(name="wv1p", bufs=1))
                wv1_sb = wv1p.tile([P, CT, VCH], F16)
                for ct in range(CT):
                    nc.sync.dma_start(
                        out=wv1_sb[:, ct, :], in_=wvT_r[:, ct, VCH : 2 * VCH]
                    )
                # prefetch all kv blocks for the V-proj second half now
                kvbp = p2.enter_context(tc.tile_pool(name="kvbp", bufs=KT))
                kvb_tiles = []
                for vt in range(KT):
                    kvb = kvbp.tile([P, CT, P], F16, tag="kvb", name="kvb")
                    nc.sync.dma_start(
                        out=kvb, in_=kvT_r[:, :, vt * P : (vt + 1) * P]
                    )
                    kvb_tiles.append(kvb)
                ptp = p2.enter_context(tc.tile_pool(name="ptp", bufs=kb["pt_bufs"]))
                rsp = p2.enter_context(tc.tile_pool(name="rsp", bufs=kb["rsum_bufs"]))
                sbb = p2.enter_context(tc.tile_pool(name="sbb", bufs=kb["sbb_bufs"]))
                tmpp = p2.enter_context(tc.tile_pool(name="tmpp", bufs=kb["tmp_bufs"]))
                yp = p2.enter_context(tc.tile_pool(name="yp", bufs=kb["y_bufs"]))
                psPV = p2.enter_context(
                    tc.tile_pool(name="psPV", bufs=kb["psPV_bufs"], space="PSUM")
                )
                # psSc entered last so it can be released (LIFO) before the
                # tail, freeing its 4 banks for the psO2 pool.
                psSc_ctx = ExitStack()
                psSc = psSc_ctx.enter_context(
                    tc.tile_pool(name="psSc", bufs=kb["psSc_bufs"], space="PSUM")
                )

                def emit_scores_step(ot, kt, pt01):
                    """One kt slice of a head pair's scores + exp: both heads
                    share a 4-bank PSUM tile so the Exp runs at N=2048.
                    Stationary-major matmul order (LDW dedupe); the r0/r1
                    matmuls still overlap across PE row groups on hardware."""
                    kl = slice(kt * P, (kt + 1) * P)
                    s01 = psSc.tile([P, HPT, LQ], F32, tag="ps_s", name="s01")
                    for hp in range(HPT):
                        r = slice(hp * D, (hp + 1) * D)
                        for ch in range(NCH):
                            sl = slice(ch * CH, (ch + 1) * CH)
                            nc.tensor.matmul(
                                s01[:, hp, sl], knT[r, ot, kl], qnT[r, ot, sl],
                                start=True, stop=True,
                            )
                    nc.scalar.activation(pt01[:, kt, :, :], s01, AF.Exp)

                def pv_mms(pair, hp, ch, pt01):
                    """attn@v (+softmax sum via the ones column) matmuls for
                    one (head, Lq-chunk)."""
                    sl = slice(ch * CH, (ch + 1) * CH)
                    pv = psPV.tile([D + 1, CH], F32, tag="ps_pv", name="ps_pv")
                    for kt in range(KT):
                        nc.tensor.matmul(
                            pv,
                            v_aug[:, kt, pair[0] + hp, :],
                            pt01[:, kt, hp, sl],
                            start=(kt == 0),
                            stop=(kt == KT - 1),
                        )
                    return pv

                def pv_tail_a(pv):
                    """Softmax-sum reciprocal: copy the PSUM sums row down to
                    partition 0 first (custom-DVE ops misread non-zero
                    partition bases; a plain tensor_copy handles the shift),
                    fast-recip at base 0, then DMA partition-broadcast to
                    [D, CH]."""
                    s0 = rsp.tile([1, CH], F32, tag="s0", name="s0")
                    nc.vector.tensor_copy(s0, pv[D : D + 1, :])
                    rs = rsp.tile([1, CH], F32, tag="rsum", name="rs")
                    nc.vector.reciprocal_approx_fast(rs, s0)
                    sb_b = sbb.tile([D, CH], F32, tag="sb_b", name="sb_b")
                    nc.sync.dma_start(
                        out=sb_b,
                        in_=rs[0:1, None, :].to_broadcast((1, D, CH)),
                    )
                    return sb_b

                def pv_tail_b(h, ch, pv, sb_b):
                    """Normalize into oT (DMA for the upper partition half)."""
                    par, ot = h % HPT, h // HPT
                    sl = slice(ch * CH, (ch + 1) * CH)
                    rows = slice(par * D, (par + 1) * D)
                    if par == 0:
                        nc.vector.tensor_mul(oT[rows, ot, sl], pv[0:D, :], sb_b)
                    else:
                        tmp = tmpp.tile([D, CH], BF16, tag="tmp", name="tmp")
                        nc.vector.tensor_mul(tmp, pv[0:D, :], sb_b)
                        nc.sync.dma_start(out=oT[rows, ot, sl], in_=tmp)

                def emit_vproj2(vt):
                    """Second-half V projection (heads HPC..2*HPC-1) as PE
                    filler in early pairs; kv block prefetched from DRAM."""
                    pv = psPV.tile([P, VCH], F32, tag="ps_pv", name="pv2")
                    for ct in range(CT):
                        nc.tensor.matmul(
                            pv,
                            kvb_tiles[vt][:, ct, :],
                            wv1_sb[:, ct, :],
                            start=(ct == 0),
                            stop=(ct == CT - 1),
                        )
                    nc.vector.tensor_copy(
                        v_aug[:, vt, HPC : 2 * HPC, 0:D],
                        pv.rearrange("p (h d) -> p h d", d=D),
                    )

                def emit_oproj(u, ct0, ct1, mode, pool=None):
                    """Partial O-projection over ct0..ct1-1 for unit u.
                    mode: 'init' writes y_mid, 'accum' adds to it, 'final'
                    adds the last partial and DMAs the row out.
                    The pair-7-interleaved finals pass their own pool (carved
                    from the freed scores banks) so they never clobber
                    in-flight PV tiles in the ps_pv ring."""
                    yt, vch = divmod(u, NVCH)
                    sl = slice(vch * VCH, (vch + 1) * VCH)
                    ps = (pool or psPV).tile([P, VCH], F32, tag="ps_pv", name="ps_o")
                    for ct in range(ct0, ct1):
                        nc.tensor.matmul(
                            ps,
                            oT[:, ct, yt * P : (yt + 1) * P],
                            wp_sb[:, ct, sl],
                            start=(ct == ct0),
                            stop=(ct == ct1 - 1),
                        )
                    if mode == "init":
                        nc.vector.tensor_copy(y_mid[:, yt, sl], ps)
                    elif mode == "accum":
                        nc.vector.tensor_add(y_mid[:, yt, sl], ps, y_mid[:, yt, sl])
                    else:
                        y_sb = yp.tile([P, VCH], F32, tag="y_sb", name="y_sb")
                        nc.vector.tensor_add(y_sb, ps, y_mid[:, yt, sl])
                        nc.sync.dma_start(out=y_r[:, yt, sl], in_=y_sb)

                NPAIR = H // 2
                nunits = (LQ // P) * NVCH      # 16 O-proj units per ct-range

                _psO2_box = [None]

                def get_psO2():
                    return _psO2_box[0]

                # PE filler per pair (keeps the HAM clock-gate warm while the
                # ACT engine works through the Exp stream):
                #   pair 0-1:  V-proj second half (6 + 2 lkv tiles)
                #   pairs 2-4: O-proj ct 0-2 init   (needs pairs 0-1 done)
                #   pairs 5-6: O-proj ct 2-4 accum  (needs pairs 2-3 done)
                #   pair 7:    O-proj ct 4-6 accum  (needs pairs 4-5 done)
                #   tail:      PV(pair 7) + O-proj ct 6-8 + y writeout
                filler = {pi: [] for pi in range(NPAIR)}
                for vt in range(KT):
                    filler[min(vt // 6, 1)].append(lambda vt=vt: emit_vproj2(vt))
                for u in range(nunits):
                    filler[2 + u // 6].append(
                        lambda u=u: emit_oproj(u, 0, 2, "init")
                    )
                    filler[5 + u // 8].append(
                        lambda u=u: emit_oproj(u, 2, 4, "accum")
                    )
                    filler[7].append(
                        lambda u=u: emit_oproj(u, 4, 6, "accum")
                    )

                def pv_steps_for(pair, pt01, ch_major=False, extra_by_unit=None):
                    """PV units as a 3-stage software pipeline: unit j's
                    recip+broadcast (tail_a) is emitted after unit j+1's
                    matmuls, its normalize (tail_b) after unit j+2's, so the
                    DVE/DMA chain hides under the PE stream.
                    extra_by_unit: {unit_idx: [callables]} appended right
                    after that unit's tail_b (used to interleave the final
                    O-proj units into pair 7)."""
                    if ch_major:
                        units = [(hp, ch) for ch in range(NCH)
                                 for hp in range(HPT)]
                    else:
                        units = [(hp, ch) for hp in range(HPT)
                                 for ch in range(NCH)]
                    n = len(units)
                    pvs = [None] * n
                    sbs = [None] * n
                    steps = []

                    def mk_mms(j):
                        def f():
                            hp, ch = units[j]
                            pvs[j] = pv_mms(pair, hp, ch, pt01)
                        return f

                    def mk_tail_a(j):
                        def f():
                            sbs[j] = pv_tail_a(pvs[j])
                        return f

                    def mk_tail_b(j):
                        def f():
                            hp, ch = units[j]
                            pv_tail_b(pair[0] + hp, ch, pvs[j], sbs[j])
                        return f

                    for i in range(n + 2):
                        if i < n:
                            steps.append(mk_mms(i))
                        if 0 <= i - 1 < n:
                            steps.append(mk_tail_a(i - 1))
                        if 0 <= i - 2 < n:
                            steps.append(mk_tail_b(i - 2))
                            if extra_by_unit and (i - 2) in extra_by_unit:
                                steps.extend(extra_by_unit[i - 2])
                    return steps

                pend = None   # steps of the previous pair's PV work
                for pi in range(NPAIR):
                    pair = (2 * pi, 2 * pi + 1)
                    ot = pi
                    pt01 = ptp.tile([P, KT, HPT, LQ], BF16, tag="pt", name="pt01")
                    psteps = (pend or []) + filler[pi]
                    np_done = 0
                    for kt in range(KT):
                        emit_scores_step(ot, kt, pt01)
                        want = (kt + 1) * len(psteps) // KT
                        while np_done < want:
                            psteps[np_done]()
                            np_done += 1
                    while np_done < len(psteps):
                        psteps[np_done]()
                        np_done += 1
                    if pi < NPAIR - 1:
                        pend = pv_steps_for(pair, pt01)
                    else:
                        # Pair 7: ch-major PV units; interleave the final
                        # O-proj units as soon as their token block's oT is
                        # complete (ch0 tails done -> yt 0-3, ch1 -> yt 4-7).
                        extra = {
                            1: [lambda u=u: emit_oproj(u, 6, CT, "final",
                                                       pool=get_psO2())
                                for u in range(0, 8)],
                            3: [lambda u=u: emit_oproj(u, 6, CT, "final",
                                                       pool=get_psO2())
                                for u in range(8, nunits)],
                        }
                        pend = pv_steps_for(
                            pair, pt01, ch_major=True, extra_by_unit=extra
                        )
                # Scores are done; free the 4 psSc banks and run the tail
                # (pair 7 PV + interleaved ct 6-8 finals) with a dedicated
                # O-proj pool carved out of the freed space.
                psSc_ctx.close()
                psO2 = p2.enter_context(
                    tc.tile_pool(name="psO2", bufs=3, space="PSUM")
                )
                _psO2_box[0] = psO2
                for s in pend:
                    s()

    nc.finalize()
    return nc


_NC_CACHE = {}


def _get_nc(C, H, LQ, LKV, knobs=None):
    key = (C, H, LQ, LKV, tuple(sorted((knobs or {}).items())))
    if key not in _NC_CACHE:
        _NC_CACHE[key] = build_nc(C, H, LQ, LKV, knobs=knobs)
    return _NC_CACHE[key]


def _host_inputs(q, kv, Wq, Wkv, Wproj, bproj, tau, H):
    B, LQ, C = q.shape
    P, D = 128, C // H
    HPT = P // D

    f16 = lambda a: np.ascontiguousarray(
        np.asarray(a, dtype=np.float32).astype(np.float16)
    )
    bf16 = lambda a: np.ascontiguousarray(
        np.asarray(a, dtype=np.float32).astype(ml_dtypes.bfloat16)
    )

    wqT = f16(np.asarray(Wq).T)
    wkT = f16(np.asarray(Wkv)[:C].T)
    wvT = f16(np.asarray(Wkv)[C:].T)
    wpT = bf16(np.asarray(Wproj).T)
    # kernel computes rk = sqrt((1/ssq) * itau2), so pass 1/tau^2 here
    tau2 = np.full((HPT, 1), float(np.asarray(tau)) ** -2, dtype=np.float32)
    ones_blk = np.zeros((P, HPT), dtype=np.float16)
    for p in range(P):
        ones_blk[p, p // D] = 1.0
    blk2 = np.ascontiguousarray(ones_blk.T)

    shared = {
        "wqT": wqT, "wkT": wkT, "wvT": wvT, "wpT": wpT,
        "tau2": tau2, "ones_blk": ones_blk, "blk2": blk2,
    }
    qn = np.asarray(q, dtype=np.float32)
    kvn = np.asarray(kv, dtype=np.float32)
    in_maps = []
    for b in range(B):
        m = dict(shared)
        m["qT"] = f16(qn[b].T)
        m["kvT"] = f16(kvn[b].T)
        in_maps.append(m)
    return in_maps


def kernel(q, kv, Wq, Wkv, Wproj, bproj, tau, _trace=False, _knobs=None):
    B, LQ, C = q.shape
    LKV = kv.shape[1]
    H = 16 if C == 1024 else max(1, C // 64)
    assert B == NCORES, f"expected B == {NCORES}, got {B}"

    nc = _get_nc(C, H, LQ, LKV, knobs=_knobs)
    in_maps = _host_inputs(q, kv, Wq, Wkv, Wproj, bproj, tau, H)
    res = run_bass_kernel_spmd(
        nc, in_maps, core_ids=list(range(NCORES)), trace=_trace
    )
    bp = np.asarray(bproj, dtype=np.float64).reshape(1, C)
    out = np.stack(
        [res.results[b]["y"].astype(np.float64) + bp for b in range(B)], axis=0
    )
    out = out.astype(np.asarray(q).dtype)
    if _trace:
        kernel._last_result = res
    return out


# revision 21
# speedup vs baseline: 1.4434x; 1.4434x over previous
"""CrossAttention (cosine-sim, learnable temperature) Trainium2 kernel, v4.

Math (per batch element b, reference in fp32):
    qh  = (q @ Wq.T)   -> [Lq, C] -> heads [H, Lq, D]
    k,v = (kv @ Wkv.T) -> k,v [H, Lkv, D]
    qn = qh / ||qh||_d; kn = k / ||k||_d
    attn = softmax(qn @ kn.T / tau); out = attn @ v
    y = out @ Wproj.T + bproj         (bproj added on host)

Distribution: pure data-parallel over B=8 across the 8 NeuronCores (one
batch element per core, weights replicated, no collectives).

v4 design notes (changes vs v2 baseline, driven by NTFF profiles):
  * Both heads of a pair share ONE 4-bank PSUM scores tile and ONE
    bf16 pt tile, so the softmax Exp runs at N=2048 instead of 2x
    N=1024 - 64 ACTIVATEs instead of 128, amortizing the ~480ns
    per-instruction overhead.  Measured: 15.7us of Exp per pair vs
    21.4us in v2.  The scores tile is single-buffered (4 banks); the
    interleaved PV/filler matmuls cover the Exp latency between kt
    steps.
  * Dummy zero-scale Sqrt/Exp activations preload the ACT table sets
    (sqrt set at kernel start, exp set during the phase boundary), so
    the first real Exp doesn't stall scores - in the v2 trace that
    stall tipped the PE into a 37us HAM re-throttle covering pairs
    0-1.
  * The norm chain and the softmax-sum normalization use PE
    ones-matmul broadcasts exactly as v2 (a v3 experiment with
    SBUF->SBUF broadcast DMAs was 2x slower: the DMA path runs at
    ~58GB/s and its queue backs up ~20us).
  * kv-ch0 input DMAs are issued before the wk columns (the first K
    job needs all kv chunks but only one wk column), cutting the DMA
    lead-in before the first matmul.
  * The phase-2 V-proj kv blocks and wv second half are DMA'd during
    phase 1 (their pools are allocated up front), so pair 0's filler
    never waits on DMA at the phase boundary.
  * O-proj filler spread over 3 partial stages (ct 0-2 at pairs 2-4,
    2-4 at 5-6, 4-6 during pair 7); the ct 6-8 finals interleave into
    pair 7's PV steps via a dedicated PSUM pool carved from the freed
    scores banks, shrinking the ACT-idle tail.
"""

import sys

sys.path.insert(0, "/opt/trn_rl_repo")

import numpy as np
import ml_dtypes

import concourse.bass as bass
import concourse.bacc as bacc
import concourse.mybir as mybir
from concourse.tile import TileContext
from concourse.bass_utils import run_bass_kernel_spmd

AF = mybir.ActivationFunctionType
F32 = mybir.dt.float32
F32R = mybir.dt.float32r
F16 = mybir.dt.float16
BF16 = mybir.dt.bfloat16

NCORES = 8


DEFAULT_KNOBS = dict(
    psA_bufs=4, psS_bufs=2, psB_bufs=2,
    sq_bufs=3, smalls_bufs=4, rbs_bufs=2,
    psSc_bufs=1, psPV_bufs=3, psBc_bufs=1,
    pt_bufs=2, rsum_bufs=2, sbb_bufs=3, tmp_bufs=2, y_bufs=2,
)


def build_nc(C=1024, H=16, LQ=1024, LKV=1024, knobs=None):
    kb = dict(DEFAULT_KNOBS)
    if knobs:
        kb.update(knobs)
    P = 128
    D = C // H            # head dim (64)
    OT = C // P           # feature tiles (8)
    CT = C // P           # contraction tiles (8)
    KT = LKV // P         # lkv partition tiles (8)
    HPT = P // D          # heads per 128-tile (2)
    CH = min(512, LQ)     # free-dim chunk per psum bank (fp32)
    NCH = LQ // CH        # chunks of Lq (2)
    VCH = min(512, C)     # chunk of output features for V projection
    NVCH = C // VCH
    HPC = VCH // D        # heads per v-projection chunk (8)

    nc = bacc.Bacc("TRN2", target_bir_lowering=False)

    qT = nc.dram_tensor("qT", [C, LQ], F16, kind="ExternalInput")
    kvT = nc.dram_tensor("kvT", [C, LKV], F16, kind="ExternalInput")
    wqT = nc.dram_tensor("wqT", [C, C], F16, kind="ExternalInput")
    wkT = nc.dram_tensor("wkT", [C, C], F16, kind="ExternalInput")
    wvT = nc.dram_tensor("wvT", [C, C], F16, kind="ExternalInput")
    wpT = nc.dram_tensor("wpT", [C, C], BF16, kind="ExternalInput")
    tau2 = nc.dram_tensor("tau2", [HPT, 1], F32, kind="ExternalInput")
    ones_blk = nc.dram_tensor("ones_blk", [P, HPT], F16, kind="ExternalInput")
    blk2 = nc.dram_tensor("blk2", [HPT, P], F16, kind="ExternalInput")
    y = nc.dram_tensor("y", [LQ, C], F32, kind="ExternalOutput")

    qT_r = qT.rearrange("(ct p) l -> p ct l", p=P)
    kvT_r = kvT.rearrange("(ct p) l -> p ct l", p=P)
    wqT_r = wqT.rearrange("(ct p) o -> p ct o", p=P)
    wkT_r = wkT.rearrange("(ct p) o -> p ct o", p=P)
    wvT_r = wvT.rearrange("(ct p) o -> p ct o", p=P)
    wpT_r = wpT.rearrange("(ct p) o -> p ct o", p=P)
    y_r = y.rearrange("(yt p) o -> p yt o", p=P)

    with TileContext(nc) as tc:
        from contextlib import ExitStack

        with ExitStack() as stk:
            # ---------- persistent pools --------------------------------
            persist = stk.enter_context(tc.tile_pool(name="persist", bufs=1))
            qnT = persist.tile([P, OT, LQ], F16)            # qh * rq
            knT = persist.tile([P, OT, LKV], F16)           # kh * rk / tau
            v_aug = persist.tile([P, KT, H, D + 1], BF16)   # [v | ones]
            oT = persist.tile([P, CT, LQ], BF16)            # (attn@v)/sum
            wp_sb = persist.tile([P, CT, C], BF16)
            consts = stk.enter_context(tc.tile_pool(name="consts", bufs=1))
            ones_blk_sb = consts.tile([P, HPT], F16)
            blk2_sb = consts.tile([HPT, P], F16)
            tau2_sb = consts.tile([HPT, 1], F32)
            ones64 = consts.tile([1, D], BF16)
            scr = consts.tile([HPT, 1], F32)

            nc.sync.dma_start(out=ones_blk_sb, in_=ones_blk[:, :])
            nc.sync.dma_start(out=blk2_sb, in_=blk2[:, :])
            nc.sync.dma_start(out=tau2_sb, in_=tau2[:, :])
            nc.vector.memset(ones64, 1.0)
            nc.vector.memset(v_aug[:, :, :, D : D + 1], 1.0)
            # Preload the sqrt ACT table set before the first real Sqrt.
            nc.scalar.activation(scr, tau2_sb, AF.Sqrt, scale=0.0)

            # Phase-2 V-proj inputs: pools allocated up front (they must
            # outlive phase 1's pools on the stack) and DMA'd during
            # phase 1, so pair 0's filler never waits at the boundary.
            wv1p = stk.enter_context(tc.tile_pool(name="wv1p", bufs=1))
            wv1_sb = wv1p.tile([P, CT, VCH], F16)
            kvbp = stk.enter_context(tc.tile_pool(name="kvbp", bufs=KT))
            kvb_tiles = []
            for vt in range(KT):
                kvb = kvbp.tile([P, CT, P], F16, tag="kvb", name="kvb")
                kvb_tiles.append(kvb)

            # ---------- phase 1 (scoped so pools free before phase 2) ----
            p1 = ExitStack()
            # kv ch0 chunks first (the first K job needs all of them but
            # only one wk column), then wk/wv columns, kv ch1, then q + q
            # weights (phase 1b), O-proj weights and phase-2 V inputs last.
            p1w = p1.enter_context(tc.tile_pool(name="p1w", bufs=1))
            kvT_sb = p1w.tile([P, CT, LKV], F16)
            wk_sb = p1w.tile([P, CT, C], F16)
            wv_sb = p1w.tile([P, CT, VCH], F16)   # first half only (vch 0)
            qT_sb = p1w.tile([P, CT, LQ], F16)
            wq_sb = p1w.tile([P, CT, C], F16)
            for ct in range(CT):
                nc.sync.dma_start(out=kvT_sb[:, ct, 0:CH], in_=kvT_r[:, ct, 0:CH])
            for ct in range(CT):
                sl = slice(ct * P, (ct + 1) * P)
                nc.sync.dma_start(out=wk_sb[:, :, sl], in_=wkT_r[:, :, sl])
            for ct in range(VCH // P):
                sl = slice(ct * P, (ct + 1) * P)
                nc.sync.dma_start(out=wv_sb[:, :, sl], in_=wvT_r[:, :, sl])
            for ct in range(CT):
                nc.sync.dma_start(
                    out=kvT_sb[:, ct, CH:LKV], in_=kvT_r[:, ct, CH:LKV]
                )
            for ct in range(CT):
                sl = slice(ct * P, (ct + 1) * P)
                nc.sync.dma_start(out=qT_sb[:, ct, :], in_=qT_r[:, ct, :])
                nc.sync.dma_start(out=wq_sb[:, :, sl], in_=wqT_r[:, :, sl])
            for ct in range(CT):
                nc.sync.dma_start(out=wp_sb[:, ct, :], in_=wpT_r[:, ct, :])
            for ct in range(CT):
                nc.sync.dma_start(
                    out=wv1_sb[:, ct, :], in_=wvT_r[:, ct, VCH : 2 * VCH]
                )
            for vt in range(KT):
                nc.sync.dma_start(
                    out=kvb_tiles[vt], in_=kvT_r[:, :, vt * P : (vt + 1) * P]
                )

            # ============ PHASE 1a: K norm-proj + V proj ================
            class Job:
                def A(self):
                    pass

                def B(self):
                    pass

                def Cs(self):
                    pass

            def run_pipeline(jobs):
                n = len(jobs)
                for i in range(n + 2):
                    if i < n:
                        jobs[i].A()
                    if 0 <= i - 1 < n:
                        jobs[i - 1].B()
                    if 0 <= i - 2 < n:
                        jobs[i - 2].Cs()

            with ExitStack() as p1c:
                sqp = p1c.enter_context(tc.tile_pool(name="sqp", bufs=kb["sq_bufs"]))
                smalls = p1c.enter_context(
                    tc.tile_pool(name="smalls", bufs=kb["smalls_bufs"])
                )
                rbs = p1c.enter_context(tc.tile_pool(name="rbsa", bufs=kb["rbs_bufs"]))
                psA = p1c.enter_context(
                    tc.tile_pool(name="psA", bufs=kb["psA_bufs"], space="PSUM")
                )
                psS = p1c.enter_context(
                    tc.tile_pool(name="psS", bufs=kb["psS_bufs"], space="PSUM")
                )
                psB = p1c.enter_context(
                    tc.tile_pool(name="psB", bufs=kb["psB_bufs"], space="PSUM")
                )

                class NormJob(Job):
                    """Shared K/Q norm-projection job body (v2 chain)."""

                    def __init__(self, ot, ch):
                        self.ot, self.ch = ot, ch
                        self.sl = slice(ch * CH, (ch + 1) * CH)

                    def A(self):
                        self.ph = psA.tile([P, CH], F32, tag="ph", name="ph")
                        w_sb, x_sb = self.srcs()
                        wcol = w_sb[:, :, self.ot * P : (self.ot + 1) * P]
                        for ct in range(CT):
                            nc.tensor.matmul(
                                self.ph,
                                wcol[:, ct, :],
                                x_sb[:, ct, self.sl],
                                start=(ct == 0),
                                stop=(ct == CT - 1),
                            )
                        self.sq = sqp.tile([P, CH], F16, tag="sq", name="sq")
                        nc.scalar.activation(self.sq, self.ph, AF.Square)

                    def B(self):
                        ssq = psS.tile([HPT, CH], F32, tag="ssq", name="ssq")
                        nc.tensor.matmul(ssq, ones_blk_sb, self.sq, start=True, stop=True)
                        # rr = sqrt(ssq * scale) in f16, so the broadcast
                        # matmul below runs at full f16 rate
                        self.rr = smalls.tile([HPT, CH], F16, tag="rr", name="rr")
                        nc.scalar.activation(
                            self.rr, ssq, AF.Sqrt, scale=self.sqrt_scale()
                        )

                    def Cs(self):
                        rb = psB.tile([P, CH], F32, tag="rb", name="rb")
                        nc.tensor.matmul(rb, blk2_sb, self.rr, start=True, stop=True)
                        rb_sb = rbs.tile([P, CH], F32, tag="rb_sb", name="rb_sb")
                        nc.vector.reciprocal_approx_fast(rb_sb, rb)
                        nc.vector.tensor_mul(
                            self.dst()[:, self.ot, self.sl], self.ph, rb_sb
                        )

                class KJob(NormJob):
                    def srcs(self):
                        return wk_sb, kvT_sb

                    def sqrt_scale(self):
                        return tau2_sb

                    def dst(self):
                        return knT

                class QJob(NormJob):
                    def srcs(self):
                        return wq_sb, qT_sb

                    def sqrt_scale(self):
                        return 1.0

                    def dst(self):
                        return qnT

                class VJob(Job):
                    def __init__(self, vch, vt):
                        self.vch, self.vt = vch, vt

                    def A(self):
                        self.pv = psA.tile([P, VCH], F32, tag="ph", name="pv")
                        wcol = wv_sb[:, :, self.vch * VCH : (self.vch + 1) * VCH]
                        for ct in range(CT):
                            nc.tensor.matmul(
                                self.pv,
                                kvT_sb[:, ct, self.vt * P : (self.vt + 1) * P],
                                wcol[:, ct, :],
                                start=(ct == 0),
                                stop=(ct == CT - 1),
                            )

                    def Cs(self):
                        nc.vector.tensor_copy(
                            v_aug[
                                :, self.vt, self.vch * HPC : (self.vch + 1) * HPC, 0:D
                            ],
                            self.pv.rearrange("p (h d) -> p h d", d=D),
                        )

                # One merged pipeline: ch0 K jobs first (they only need the
                # ch0 kv halves), V jobs slotted in as their inputs land,
                # then ch1 K jobs, then all Q jobs.  A single pool scope
                # means no pipeline drain at the K/Q boundary.
                jobs = [KJob(ot, 0) for ot in range(4)]
                for i in range(4):
                    jobs += [KJob(4 + i, 0), VJob(0, i)]
                for i in range(4):
                    jobs += [KJob(i, 1), VJob(0, 4 + i)]
                jobs += [KJob(4 + i, 1) for i in range(4)]
                jobs += [QJob(i // 2, i % 2) for i in range(2 * OT)]
                run_pipeline(jobs)

            # Preload the ACT exp table set during the phase boundary so the
            # first real Exp doesn't stall the (single-buffered) scores tile.
            nc.scalar.activation(scr, tau2_sb, AF.Exp, scale=0.0)

            # free phase-1 inputs/weights before the big pt pool allocates
            p1.close()

            # ============ PHASE 2: attention (head pairs) ===============
            with ExitStack() as p2:
                ymp = p2.enter_context(tc.tile_pool(name="ymp", bufs=1))
                y_mid = ymp.tile([P, LQ // P, C], BF16)
                ptp = p2.enter_context(tc.tile_pool(name="ptp", bufs=kb["pt_bufs"]))
                rsp = p2.enter_context(tc.tile_pool(name="rsp", bufs=kb["rsum_bufs"]))
                sbb = p2.enter_context(tc.tile_pool(name="sbb", bufs=kb["sbb_bufs"]))
                tmpp = p2.enter_context(tc.tile_pool(name="tmpp", bufs=kb["tmp_bufs"]))
                yp = p2.enter_context(tc.tile_pool(name="yp", bufs=kb["y_bufs"]))
                psPV = p2.enter_context(
                    tc.tile_pool(name="psPV", bufs=kb["psPV_bufs"], space="PSUM")
                )
                psBc = p2.enter_context(
                    tc.tile_pool(name="psBc", bufs=kb["psBc_bufs"], space="PSUM")
                )
                # psSc entered last so it can be released (LIFO) before the
                # tail, freeing its 4 banks for the psO2 pool.
                psSc_ctx = ExitStack()
                psSc = psSc_ctx.enter_context(
                    tc.tile_pool(name="psSc", bufs=kb["psSc_bufs"], space="PSUM")
                )

                def emit_scores_step(ot, kt, pt01):
                    """One kt slice of a head pair's scores + exp: both heads
                    share a 4-bank PSUM tile so the Exp runs at N=2048."""
                    kl = slice(kt * P, (kt + 1) * P)
                    s01 = psSc.tile([P, HPT, LQ], F32, tag="ps_s", name="s01")
                    for hp in range(HPT):
                        r = slice(hp * D, (hp + 1) * D)
                        for ch in range(NCH):
                            sl = slice(ch * CH, (ch + 1) * CH)
                            nc.tensor.matmul(
                                s01[:, hp, sl], knT[r, ot, kl], qnT[r, ot, sl],
                                start=True, stop=True,
                            )
                    nc.scalar.activation(pt01[:, kt, :, :], s01, AF.Exp)

                def pv_mms(pair, hp, ch, pt01):
                    """attn@v (+softmax sum via the ones column) matmuls for
                    one (head, Lq-chunk)."""
                    sl = slice(ch * CH, (ch + 1) * CH)
                    pv = psPV.tile([D + 1, CH], F32, tag="ps_pv", name="ps_pv")
                    for kt in range(KT):
                        nc.tensor.matmul(
                            pv,
                            v_aug[:, kt, pair[0] + hp, :],
                            pt01[:, kt, hp, sl],
                            start=(kt == 0),
                            stop=(kt == KT - 1),
                        )
                    return pv

                def pv_tail(h, ch, pv):
                    """Softmax-sum fast-recip after a PE ones-broadcast,
                    then normalize into oT (v2 chain)."""
                    par, ot = h % HPT, h // HPT
                    sl = slice(ch * CH, (ch + 1) * CH)
                    sums = rsp.tile([1, CH], BF16, tag="rsum", name="sums")
                    nc.vector.tensor_copy(sums, pv[D : D + 1, :])
                    ps_b = psBc.tile([D, CH], F32, tag="ps_b", name="ps_b")
                    nc.tensor.matmul(ps_b, ones64, sums, start=True, stop=True)
                    sb_b = sbb.tile([D, CH], F32, tag="sb_b", name="sb_b")
                    nc.vector.reciprocal_approx_fast(sb_b, ps_b)
                    rows = slice(par * D, (par + 1) * D)
                    if par == 0:
                        nc.vector.tensor_mul(oT[rows, ot, sl], pv[0:D, :], sb_b)
                    else:
                        tmp = tmpp.tile([D, CH], BF16, tag="tmp", name="tmp")
                        nc.vector.tensor_mul(tmp, pv[0:D, :], sb_b)
                        nc.sync.dma_start(out=oT[rows, ot, sl], in_=tmp)

                def emit_vproj2(vt):
                    """Second-half V projection (heads HPC..2*HPC-1) as PE
                    filler in early pairs; kv block prefetched in phase 1."""
                    pv = psPV.tile([P, VCH], F32, tag="ps_pv", name="pv2")
                    for ct in range(CT):
                        nc.tensor.matmul(
                            pv,
                            kvb_tiles[vt][:, ct, :],
                            wv1_sb[:, ct, :],
                            start=(ct == 0),
                            stop=(ct == CT - 1),
                        )
                    nc.vector.tensor_copy(
                        v_aug[:, vt, HPC : 2 * HPC, 0:D],
                        pv.rearrange("p (h d) -> p h d", d=D),
                    )

                def emit_oproj(u, ct0, ct1, mode, pool=None):
                    """Partial O-projection over ct0..ct1-1 for unit u.
                    mode: 'init' writes y_mid, 'accum' adds to it, 'final'
                    adds the last partial and DMAs the row out.
                    The pair-7-interleaved finals pass their own pool (carved
                    from the freed scores banks) so they never clobber
                    in-flight PV tiles in the ps_pv ring."""
                    yt, vch = divmod(u, NVCH)
                    sl = slice(vch * VCH, (vch + 1) * VCH)
                    ps = (pool or psPV).tile([P, VCH], F32, tag="ps_pv", name="ps_o")
                    for ct in range(ct0, ct1):
                        nc.tensor.matmul(
                            ps,
                            oT[:, ct, yt * P : (yt + 1) * P],
                            wp_sb[:, ct, sl],
                            start=(ct == ct0),
                            stop=(ct == ct1 - 1),
                        )
                    if mode == "init":
                        nc.vector.tensor_copy(y_mid[:, yt, sl], ps)
                    elif mode == "accum":
                        nc.vector.tensor_add(y_mid[:, yt, sl], ps, y_mid[:, yt, sl])
                    else:
                        y_sb = yp.tile([P, VCH], F32, tag="y_sb", name="y_sb")
                        nc.vector.tensor_add(y_sb, ps, y_mid[:, yt, sl])
                        nc.sync.dma_start(out=y_r[:, yt, sl], in_=y_sb)

                NPAIR = H // 2
                nunits = (LQ // P) * NVCH      # 16 O-proj units per ct-range

                _psO2_box = [None]

                def get_psO2():
                    return _psO2_box[0]

                # PE filler per pair (keeps the HAM clock-gate warm while the
                # ACT engine works through the Exp stream):
                #   pair 0-1:  V-proj second half (6 + 2 lkv tiles)
                #   pairs 2-4: O-proj ct 0-2 init   (needs pairs 0-1 done)
                #   pairs 5-6: O-proj ct 2-4 accum  (needs pairs 2-3 done)
                #   pair 7:    O-proj ct 4-6 accum  (needs pairs 4-5 done)
                #   tail:      PV(pair 7) + O-proj ct 6-8 + y writeout
                filler = {pi: [] for pi in range(NPAIR)}
                for vt in range(KT):
                    filler[min(vt // 6, 1)].append(lambda vt=vt: emit_vproj2(vt))
                for u in range(nunits):
                    filler[2 + u // 6].append(
                        lambda u=u: emit_oproj(u, 0, 2, "init")
                    )
                    filler[5 + u // 8].append(
                        lambda u=u: emit_oproj(u, 2, 4, "accum")
                    )
                    filler[7].append(
                        lambda u=u: emit_oproj(u, 4, 6, "accum")
                    )

                def pv_steps_for(pair, pt01, ch_major=False, extra_by_unit=None):
                    """PV units software-pipelined: unit j's (DVE-gated) tail
                    is emitted after unit j+1's matmuls so the in-order PE
                    queue never waits on the sum-reciprocal chain.
                    extra_by_unit: {unit_idx: [callables]} appended right
                    after that unit's tail (used to interleave the final
                    O-proj units into pair 7)."""
                    if ch_major:
                        units = [(hp, ch) for ch in range(NCH)
                                 for hp in range(HPT)]
                    else:
                        units = [(hp, ch) for hp in range(HPT)
                                 for ch in range(NCH)]
                    n = len(units)
                    pvs = [None] * n
                    steps = []

                    def mk_mms(j):
                        def f():
                            hp, ch = units[j]
                            pvs[j] = pv_mms(pair, hp, ch, pt01)
                        return f

                    def mk_tail(j):
                        def f():
                            hp, ch = units[j]
                            pv_tail(pair[0] + hp, ch, pvs[j])
                        return f

                    for i in range(n + 1):
                        if i < n:
                            steps.append(mk_mms(i))
                        if 0 <= i - 1 < n:
                            steps.append(mk_tail(i - 1))
                            if extra_by_unit and (i - 1) in extra_by_unit:
                                steps.extend(extra_by_unit[i - 1])
                    return steps

                pend = None   # steps of the previous pair's PV work
                for pi in range(NPAIR):
                    pair = (2 * pi, 2 * pi + 1)
                    ot = pi
                    pt01 = ptp.tile([P, KT, HPT, LQ], BF16, tag="pt", name="pt01")
                    psteps = (pend or []) + filler[pi]
                    np_done = 0
                    for kt in range(KT):
                        emit_scores_step(ot, kt, pt01)
                        want = (kt + 1) * len(psteps) // KT
                        while np_done < want:
                            psteps[np_done]()
                            np_done += 1
                    while np_done < len(psteps):
                        psteps[np_done]()
                        np_done += 1
                    if pi < NPAIR - 1:
                        pend = pv_steps_for(pair, pt01)
                    else:
                        # Pair 7: ch-major PV units; interleave the final
                        # O-proj units as soon as their token block's oT is
                        # complete (ch0 tails done -> yt 0-3, ch1 -> yt 4-7).
                        extra = {
                            1: [lambda u=u: emit_oproj(u, 6, CT, "final",
                                                       pool=get_psO2())
                                for u in range(0, 8)],
                            3: [lambda u=u: emit_oproj(u, 6, CT, "final",
                                                       pool=get_psO2())
                                for u in range(8, nunits)],
                        }
                        pend = pv_steps_for(
                            pair, pt01, ch_major=True, extra_by_unit=extra
                        )
                # Scores are done; free the 4 psSc banks and run the tail
                # (pair 7 PV + interleaved ct 6-8 finals) with a dedicated
                # O-proj pool carved out of the freed space.
                psSc_ctx.close()
                psO2 = p2.enter_context(
                    tc.tile_pool(name="psO2", bufs=3, space="PSUM")
                )
                _psO2_box[0] = psO2
                for s in pend:
                    s()

    nc.finalize()
    return nc


_NC_CACHE = {}


def _get_nc(C, H, LQ, LKV, knobs=None):
    key = (C, H, LQ, LKV, tuple(sorted((knobs or {}).items())))
    if key not in _NC_CACHE:
        _NC_CACHE[key] = build_nc(C, H, LQ, LKV, knobs=knobs)
    return _NC_CACHE[key]


def _host_inputs(q, kv, Wq, Wkv, Wproj, bproj, tau, H):
    B, LQ, C = q.shape
    P, D = 128, C // H
    HPT = P // D

    f16 = lambda a: np.ascontiguousarray(
        np.asarray(a, dtype=np.float32).astype(np.float16)
    )
    bf16 = lambda a: np.ascontiguousarray(
        np.asarray(a, dtype=np.float32).astype(ml_dtypes.bfloat16)
    )

    wqT = f16(np.asarray(Wq).T)
    wkT = f16(np.asarray(Wkv)[:C].T)
    wvT = f16(np.asarray(Wkv)[C:].T)
    wpT = bf16(np.asarray(Wproj).T)
    tau2 = np.full((HPT, 1), float(np.asarray(tau)) ** 2, dtype=np.float32)
    ones_blk = np.zeros((P, HPT), dtype=np.float16)
    for p in range(P):
        ones_blk[p, p // D] = 1.0
    blk2 = np.ascontiguousarray(ones_blk.T)

    shared = {
        "wqT": wqT, "wkT": wkT, "wvT": wvT, "wpT": wpT,
        "tau2": tau2, "ones_blk": ones_blk, "blk2": blk2,
    }
    qn = np.asarray(q, dtype=np.float32)
    kvn = np.asarray(kv, dtype=np.float32)
    in_maps = []
    for b in range(B):
        m = dict(shared)
        m["qT"] = f16(qn[b].T)
        m["kvT"] = f16(kvn[b].T)
        in_maps.append(m)
    return in_maps


def kernel(q, kv, Wq, Wkv, Wproj, bproj, tau, _trace=False, _knobs=None):
    B, LQ, C = q.shape
    LKV = kv.shape[1]
    H = 16 if C == 1024 else max(1, C // 64)
    assert B == NCORES, f"expected B == {NCORES}, got {B}"

    nc = _get_nc(C, H, LQ, LKV, knobs=_knobs)
    in_maps = _host_inputs(q, kv, Wq, Wkv, Wproj, bproj, tau, H)
    res = run_bass_kernel_spmd(
        nc, in_maps, core_ids=list(range(NCORES)), trace=_trace
    )
    bp = np.asarray(bproj, dtype=np.float64).reshape(1, C)
    out = np.stack(
        [res.results[b]["y"].astype(np.float64) + bp for b in range(B)], axis=0
    )
    out = out.astype(np.asarray(q).dtype)
    if _trace:
        kernel._last_result = res
    return out
